# revision 32
# baseline (speedup 1.0000x reference)
"""Trainium2 Bass kernel for a small single-head transformer block.

Math (see reference):
  q,k per-token dot product reduces to a bilinear form:
      scores = x^T (Wq^T Wk / sqrt(D)) x  -> one GEMM (u = x @ M^T) + rowwise dot
  so the block is 4 token-parallel GEMMs (u, v, W1, W2) + masked softmax over
  L=5 + 2 layernorms, all batch-parallel across 8 cores.

Layout: samples on SBUF partitions, (l, d) in the free dim, so softmax(L) and
LN(D) are free-axis ops.  GEMM operands need the contraction dim (d) on
partitions; x arrives pre-transposed from the host, n1/h are transposed on the
tensor engine.  All matmuls run in fp16 (1 cyc/row vs 4 for fp32) with fp32
PSUM accumulation; per-feature GEMM biases (only when nonzero) are injected
into PSUM via a K=2 ones-row matmul carrying a hi/lo fp16 split of the bias.

The per-tile emission is phase-shifted (tile i+1's u/v GEMMs are emitted
before tile i's epilogue) so the in-order PE queue never waits on the DVE
softmax/LN chain — otherwise the PE idles ~13.6us per tile and the HAM
clock-gate re-throttles it cold every tile.
"""

import numpy as np

B, L, D = 16384, 5, 1024
NCORES = 8
BLOC = B // NCORES          # samples per core
P = 128                     # samples per tile
NTILES = BLOC // P
KT = D // 128               # contraction k-tiles
NB = D // 512               # 512-wide PSUM column blocks
LN_EPS = 1e-5

_cache = {}


def _hi_lo_f16(v32):
    hi = v32.astype(np.float16)
    lo = (v32 - hi.astype(np.float32)).astype(np.float16)
    return np.stack([hi, lo], 0)  # [2, D]


def _build(apply_att_affine, apply_ff_affine, has_bias_h, has_bias_s,
           keep_pat, ntiles=NTILES):
    kept = [l for l in range(L) if keep_pat[l]]
    import concourse.bacc as bacc
    import concourse.mybir as mybir
    from concourse.tile import TileContext
    from concourse.masks import make_identity
    from contextlib import ExitStack

    f16 = mybir.dt.float16
    f32 = mybir.dt.float32
    AF = mybir.ActivationFunctionType
    OP = mybir.AluOpType
    AX = mybir.AxisListType

    nc = bacc.Bacc("TRN2", target_bir_lowering=False, debug=False,
                   num_devices=NCORES)

    # ---- DRAM I/O ----
    xbf = nc.dram_tensor("xbf", [BLOC, L * D], f16, kind="ExternalInput")
    xT = nc.dram_tensor("xT", [KT, 128, NTILES, L * P], f16,
                        kind="ExternalInput")
    wts = {
        n: nc.dram_tensor(n, [KT, 128, D], f16, kind="ExternalInput")
        for n in ("wu", "wv", "w1", "w2")
    }
    if has_bias_h:
        bias_h_d = nc.dram_tensor("bias_h", [2, D], f16, kind="ExternalInput")
    if has_bias_s:
        bias_s_d = nc.dram_tensor("bias_s", [2, D], f16, kind="ExternalInput")
    keep_d = nc.dram_tensor("keep", [1, L], f32, kind="ExternalInput")
    mneg_d = nc.dram_tensor("mneg", [1, L], f32, kind="ExternalInput")
    if apply_att_affine:
        attg_d = nc.dram_tensor("attg", [1, D], f16, kind="ExternalInput")
    if apply_ff_affine:
        ffg_d = nc.dram_tensor("ffg", [1, D], f32, kind="ExternalInput")
        ffb_d = nc.dram_tensor("ffb", [1, D], f32, kind="ExternalInput")
    out_d = nc.dram_tensor("out", [BLOC, L, D], f32, kind="ExternalOutput")

    with TileContext(nc) as tc, ExitStack() as ctx:
        const = ctx.enter_context(tc.tile_pool(name="const", bufs=1))
        px = ctx.enter_context(tc.tile_pool(name="px", bufs=2))
        pxT = ctx.enter_context(tc.tile_pool(name="pxT", bufs=2))
        pvh = ctx.enter_context(tc.tile_pool(name="pvh", bufs=2))
        pzh = ctx.enter_context(tc.tile_pool(name="pzh", bufs=2))
        pns = ctx.enter_context(tc.tile_pool(name="pns", bufs=2))
        pn1 = ctx.enter_context(tc.tile_pool(name="pn1", bufs=1))
        psm = ctx.enter_context(tc.tile_pool(name="psm", bufs=3))
        pout = ctx.enter_context(tc.tile_pool(name="pout", bufs=2))
        pps = ctx.enter_context(tc.tile_pool(name="pps", bufs=8, space="PSUM"))

        # ---- constants / weights (resident) ----
        # wu/wv are needed for tile 0 immediately; w1/w2 loads are emitted
        # after tile 0's u/v GEMMs so they don't delay the first matmul.
        w_sb = {n: const.tile([128, KT, D], f16, tag=n, name=n)
                for n in ("wu", "wv", "w1", "w2")}
        # wu/wv loads are emitted inside emit_uv(0), after tile 0's x/xT
        # DMAs, per-k — so the first k=0 matmul only waits on two small DMAs.
        if has_bias_h:
            bias_h = const.tile([2, D], f16, tag="bias_h")
            nc.sync.dma_start(out=bias_h, in_=bias_h_d[:, :])
        if has_bias_s:
            bias_s = const.tile([2, D], f16, tag="bias_s")
            nc.sync.dma_start(out=bias_s, in_=bias_s_d[:, :])
        if has_bias_h or has_bias_s:
            ones2 = const.tile([2, 128], f16, tag="ones2")
            nc.vector.memset(ones2, 1.0)
        ident = const.tile([128, 128], f16, tag="ident")
        make_identity(nc, ident)
        eps_t = const.tile([128, 1], f32, tag="eps")
        nc.vector.memset(eps_t, LN_EPS)
        keep_b = const.tile([128, L], f32, tag="keep")
        nc.gpsimd.dma_start(out=keep_b, in_=keep_d[:, :].to_broadcast([128, L]))
        mneg_b = const.tile([128, L], f32, tag="mneg")
        nc.gpsimd.dma_start(out=mneg_b, in_=mneg_d[:, :].to_broadcast([128, L]))
        if apply_att_affine:
            attg_b = const.tile([128, D], f16, tag="attg")
            nc.gpsimd.dma_start(out=attg_b,
                                in_=attg_d[:, :].to_broadcast([128, D]))
        if apply_ff_affine:
            ffg_b = const.tile([128, D], f32, tag="ffg")
            nc.gpsimd.dma_start(out=ffg_b,
                                in_=ffg_d[:, :].to_broadcast([128, D]))
            ffb_b = const.tile([128, D], f32, tag="ffb")
            nc.gpsimd.dma_start(out=ffb_b,
                                in_=ffb_d[:, :].to_broadcast([128, D]))

        state = {}

        def emit_uv(i):
            """DMA x/xT for tile i, u&v GEMMs, raw scores, v eviction."""
            x_t = px.tile([128, L, D], f16, tag="x")
            nc.sync.dma_start(out=x_t,
                              in_=xbf[i * P:(i + 1) * P, :].rearrange(
                                  "p (l d) -> p l d", l=L))
            xT_t = pxT.tile([128, KT, L * P], f16, tag="xT")
            for k in range(KT):
                nc.sync.dma_start(out=xT_t[:, k, :], in_=xT[k, :, i, :])
                if i == 0:
                    for n in ("wu", "wv"):
                        nc.sync.dma_start(out=w_sb[n][:, k, :],
                                          in_=wts[n][k, :, :])

            v_sb = pvh.tile([128, L, D], f16, tag="vh")
            sc2 = psm.tile([128, L, NB], f32, tag="sc2")
            nc.vector.memset(sc2, 0.0)  # masked l slots stay 0

            for l in kept:
                lhs = [xT_t[:, k, l * P:(l + 1) * P] for k in range(KT)]
                for half, wname in ((0, "wu"), (1, "wv")):
                    for nb in range(NB):
                        ps = pps.tile([128, 512], f32, tag="mm")
                        for k in range(KT):
                            nc.tensor.matmul(
                                ps, lhs[k],
                                w_sb[wname][:, k, nb * 512:(nb + 1) * 512],
                                start=(k == 0), stop=(k == KT - 1))
                        if half == 0:  # u -> scores partial sums
                            scr = psm.tile([128, 512], f16, tag="scr")
                            nc.vector.tensor_mul(
                                scr, x_t[:, l, nb * 512:(nb + 1) * 512], ps)
                            nc.vector.reduce_sum(
                                sc2[:, l, nb:nb + 1], scr, axis=AX.X)
                        else:  # v -> SBUF
                            nc.scalar.activation(
                                out=v_sb[:, l, nb * 512:(nb + 1) * 512],
                                in_=ps, func=AF.Copy)
            state[i] = (x_t, v_sb, sc2)

        def emit_rest(i):
            """Softmax, z, LN1, transposes, FFN, LN2, output for tile i."""
            x_t, v_sb, sc2 = state.pop(i)

            # ---- masked softmax over L ----
            ssum = psm.tile([128, L], f32, tag="ssum")
            nc.vector.tensor_add(ssum, sc2[:, :, 0], sc2[:, :, 1])
            scm = psm.tile([128, L], f32, tag="scm")
            nc.vector.tensor_mul(scm, ssum, keep_b)
            nc.vector.tensor_add(scm, scm, mneg_b)
            mx = psm.tile([128, 1], f32, tag="mx")
            nc.vector.reduce_max(mx, scm, axis=AX.X)
            nmx = psm.tile([128, 1], f32, tag="nmx")
            nc.vector.tensor_scalar(out=nmx, in0=mx, scalar1=-1.0,
                                    scalar2=None, op0=OP.mult)
            e_t = psm.tile([128, L], f32, tag="e")
            nc.scalar.activation(e_t, scm, AF.Exp, bias=nmx, scale=1.0)
            den = psm.tile([128, 1], f32, tag="den")
            nc.vector.reduce_sum(den, e_t, axis=AX.X)
            rden = psm.tile([128, 1], f32, tag="rden")
            nc.vector.reciprocal(rden, den)
            probs = psm.tile([128, L], f32, tag="probs")
            nc.vector.tensor_scalar_mul(probs, e_t, rden)

            # ---- z = probs*v + x (masked l: probs==0 -> z = x) ; LN1 ----
            z_t = pzh.tile([128, L, D], f16, tag="zhT")
            for l in range(L):
                if l in kept:
                    nc.vector.tensor_scalar_mul(z_t[:, l, :], v_sb[:, l, :],
                                                probs[:, l:l + 1])
                    nc.vector.tensor_add(z_t[:, l, :], z_t[:, l, :],
                                         x_t[:, l, :])
                else:
                    nc.any.tensor_copy(out=z_t[:, l, :], in_=x_t[:, l, :])

            mv1 = psm.tile([128, L, 2], f32, tag="mv1")
            st1 = psm.tile([128, 2, 6], f32, tag="st1")
            for l in range(L):
                for c in range(2):
                    nc.vector.bn_stats(st1[:, c, :],
                                       z_t[:, l, c * 512:(c + 1) * 512])
                nc.vector.bn_aggr(mv1[:, l, :], st1)
            # rstd = exp(-0.5*ln(var+eps)): Ln+Exp share one ACT table set
            # with the softmax Exp (Sqrt would force an extra ~1.3us switch).
            la1 = psm.tile([128, L], f32, tag="la1")
            nc.scalar.activation(la1, mv1[:, :, 1], AF.Ln, bias=eps_t,
                                 scale=1.0)
            rstd1 = psm.tile([128, L], f32, tag="rstd1")
            nc.scalar.activation(rstd1, la1, AF.Exp, scale=-0.5)
            n1 = pn1.tile([128, L, D], f16, tag="n1")
            for l in range(L):
                nc.vector.tensor_scalar(
                    out=n1[:, l, :], in0=z_t[:, l, :],
                    scalar1=mv1[:, l, 0:1], scalar2=rstd1[:, l:l + 1],
                    op0=OP.subtract, op1=OP.mult)

            # ---- transpose n1 -> n1T ----
            n1T = pns.tile([128, KT, L * P], f16, tag="n1Ts")
            for l in range(L):
                tp = pps.tile([128, KT * 128], f16, tag="mm")
                for k in range(KT):
                    nc.tensor.transpose(
                        tp[:, k * 128:(k + 1) * 128],
                        n1[:, l, k * 128:(k + 1) * 128], ident)
                nc.any.tensor_copy(
                    out=n1T[:, :, l * P:(l + 1) * P],
                    in_=tp.rearrange("p (a b) -> p a b", a=KT))

            # ---- h = gelu(n1 @ W1 [+ bias_h]) ----
            h_sb = pvh.tile([128, L, D], f16, tag="vh")
            for l in range(L):
                for nb in range(NB):
                    ps = pps.tile([128, 512], f32, tag="mm")
                    if has_bias_h:
                        nc.tensor.matmul(ps, ones2,
                                         bias_h[:, nb * 512:(nb + 1) * 512],
                                         start=True, stop=False)
                    for k in range(KT):
                        nc.tensor.matmul(
                            ps, n1T[:, k, l * P:(l + 1) * P],
                            w_sb["w1"][:, k, nb * 512:(nb + 1) * 512],
                            start=(k == 0 and not has_bias_h),
                            stop=(k == KT - 1))
                    nc.scalar.activation(
                        out=h_sb[:, l, nb * 512:(nb + 1) * 512],
                        in_=ps, func=AF.Gelu)

            # ---- transpose h -> hT ----
            hT = pzh.tile([128, KT, L * P], f16, tag="zhT")
            for l in range(L):
                tp = pps.tile([128, KT * 128], f16, tag="mm")
                for k in range(KT):
                    nc.tensor.transpose(
                        tp[:, k * 128:(k + 1) * 128],
                        h_sb[:, l, k * 128:(k + 1) * 128], ident)
                nc.any.tensor_copy(
                    out=hT[:, :, l * P:(l + 1) * P],
                    in_=tp.rearrange("p (a b) -> p a b", a=KT))

            # ---- ff = h @ W2 [+ bias_s] ; s = ff + n1*att_g ; LN2 ----
            if apply_att_affine:
                n1g = pn1.tile([128, L, D], f16, tag="n1g")
                nc.vector.tensor_mul(n1g, n1, attg_b)
            else:
                n1g = n1
            s_t = pns.tile([128, L, D], f16, tag="n1Ts")
            mv2 = psm.tile([128, L, 2], f32, tag="mv2")
            for l in range(L):
                st2 = psm.tile([128, 2, 6], f32, tag="st2")
                for nb in range(NB):
                    ps = pps.tile([128, 512], f32, tag="mm")
                    if has_bias_s:
                        nc.tensor.matmul(ps, ones2,
                                         bias_s[:, nb * 512:(nb + 1) * 512],
                                         start=True, stop=False)
                    for k in range(KT):
                        nc.tensor.matmul(
                            ps, hT[:, k, l * P:(l + 1) * P],
                            w_sb["w2"][:, k, nb * 512:(nb + 1) * 512],
                            start=(k == 0 and not has_bias_s),
                            stop=(k == KT - 1))
                    nc.vector.tensor_add(
                        s_t[:, l, nb * 512:(nb + 1) * 512], ps,
                        n1g[:, l, nb * 512:(nb + 1) * 512])
                    nc.vector.bn_stats(st2[:, nb, :],
                                       s_t[:, l, nb * 512:(nb + 1) * 512])
                nc.vector.bn_aggr(mv2[:, l, :], st2)
            la2 = psm.tile([128, L], f32, tag="la2")
            nc.scalar.activation(la2, mv2[:, :, 1], AF.Ln, bias=eps_t,
                                 scale=1.0)
            rstd2 = psm.tile([128, L], f32, tag="rstd2")
            nc.scalar.activation(rstd2, la2, AF.Exp, scale=-0.5)

            for l in range(L):
                o_t = pout.tile([128, D], f32, tag="o")
                if apply_ff_affine:
                    n2 = psm.tile([128, D], f16, tag="n2")
                    nc.vector.tensor_scalar(
                        out=n2, in0=s_t[:, l, :],
                        scalar1=mv2[:, l, 0:1], scalar2=rstd2[:, l:l + 1],
                        op0=OP.subtract, op1=OP.mult)
                    nc.vector.tensor_mul(o_t, n2, ffg_b)
                    nc.vector.tensor_add(o_t, o_t, ffb_b)
                else:
                    nc.vector.tensor_scalar(
                        out=o_t, in0=s_t[:, l, :],
                        scalar1=mv2[:, l, 0:1], scalar2=rstd2[:, l:l + 1],
                        op0=OP.subtract, op1=OP.mult)
                nc.sync.dma_start(out=out_d[i * P:(i + 1) * P, l, :], in_=o_t)

        emit_uv(0)
        for n in ("w1", "w2"):
            nc.sync.dma_start(out=w_sb[n],
                              in_=wts[n][:, :, :].rearrange("k p n -> p k n"))
        for i in range(1, ntiles):
            emit_uv(i)
            emit_rest(i - 1)
        emit_rest(ntiles - 1)

    nc.compile()
    return nc


def _prep(x, mask, Wq, Wk, Wv, W1, b1, W2, b2, att_g, att_b, ff_g, ff_b):
    """Host-side preprocessing -> (flags, per-core input maps)."""
    f64 = np.float64
    apply_att_affine = not (np.all(att_g == 1.0) and np.all(att_b == 0.0))
    apply_ff_affine = not (np.all(ff_g == 1.0) and np.all(ff_b == 0.0))

    M = (Wq.astype(f64).T @ Wk.astype(f64)) / np.sqrt(np.float64(D))
    wu = np.ascontiguousarray(M.T).astype(np.float16)          # [d', d]
    wv = np.ascontiguousarray(Wv.T).astype(np.float16)         # [d, e]
    W1g = W1.astype(f64) * att_g.astype(f64)[None, :]
    w1 = np.ascontiguousarray(W1g.T).astype(np.float16)        # [d, e]
    bias_h_f = (b1.astype(f64) + W1.astype(f64) @ att_b.astype(f64)).astype(
        np.float32)
    w2 = np.ascontiguousarray(W2.T).astype(np.float16)         # [e, f]
    bias_s_f = (b2.astype(f64) + att_b.astype(f64)).astype(np.float32)
    has_bias_h = bool(np.any(bias_h_f != 0.0))
    has_bias_s = bool(np.any(bias_s_f != 0.0))

    keep = (np.all(mask != 0, axis=0)).astype(np.float32)[None, :]  # [1, L]
    keep_pat = tuple(bool(k) for k in keep[0])
    mneg = (keep - 1.0) * 30.0

    def wfmt(w):  # [D, D] -> [KT, 128, D]
        return np.ascontiguousarray(w.reshape(KT, 128, D))

    shared = dict(
        wu=wfmt(wu), wv=wfmt(wv), w1=wfmt(w1), w2=wfmt(w2),
        keep=keep, mneg=mneg)
    if has_bias_h:
        shared["bias_h"] = _hi_lo_f16(bias_h_f)
    if has_bias_s:
        shared["bias_s"] = _hi_lo_f16(bias_s_f)
    if apply_att_affine:
        shared["attg"] = att_g.astype(np.float16)[None, :]
    if apply_ff_affine:
        shared["ffg"] = ff_g.astype(np.float32)[None, :]
        shared["ffb"] = ff_b.astype(np.float32)[None, :]

    x16 = x.astype(np.float16)
    in_maps = []
    for c in range(NCORES):
        xc = x16[c * BLOC:(c + 1) * BLOC]                      # [BLOC, L, D]
        xbf = np.ascontiguousarray(xc.reshape(BLOC, L * D))
        # [i, s, l, k, dk] -> [k, dk, i, l, s]
        xTc = np.ascontiguousarray(
            xc.reshape(NTILES, P, L, KT, 128).transpose(3, 4, 0, 2, 1)
        ).reshape(KT, 128, NTILES, L * P)
        in_maps.append(dict(shared, xbf=xbf, xT=xTc))
    flags = (apply_att_affine, apply_ff_affine, has_bias_h, has_bias_s,
             keep_pat)
    return flags, in_maps


def kernel(**inputs):
    from concourse.bass_utils import run_bass_kernel_spmd

    flags, in_maps = _prep(**inputs)
    if flags not in _cache:
        _cache[flags] = _build(*flags)
    nc = _cache[flags]
    res = run_bass_kernel_spmd(nc, in_maps, core_ids=list(range(NCORES)))
    out = np.concatenate([r["out"] for r in res.results], axis=0)
    return out.astype(np.float32)


# revision 34
# speedup vs baseline: 1.0183x; 1.0183x over previous
"""Trainium2 Bass kernel for a small single-head transformer block.

Math (see reference):
  q,k per-token dot product reduces to a bilinear form:
      scores = x^T (Wq^T Wk / sqrt(D)) x  -> one GEMM (u = x @ M^T) + rowwise dot
  so the block is 4 token-parallel GEMMs (u, v, W1, W2) + masked softmax over
  L=5 + 2 layernorms, all batch-parallel across 8 cores.

Layout: samples on SBUF partitions, (l, d) in the free dim, so softmax(L) and
LN(D) are free-axis ops.  GEMM operands need the contraction dim (d) on
partitions; x arrives pre-transposed from the host, n1/h are transposed on the
tensor engine.  All matmuls run in fp16 (1 cyc/row vs 4 for fp32) with fp32
PSUM accumulation; per-feature GEMM biases (only when nonzero) are injected
into PSUM via a K=2 ones-row matmul carrying a hi/lo fp16 split of the bias.

The per-tile emission is phase-shifted (tile i+1's u/v GEMMs are emitted
before tile i's epilogue) so the in-order PE queue never waits on the DVE
softmax/LN chain — otherwise the PE idles ~13.6us per tile and the HAM
clock-gate re-throttles it cold every tile.
"""

import numpy as np

B, L, D = 16384, 5, 1024
NCORES = 8
BLOC = B // NCORES          # samples per core
P = 128                     # samples per tile
NTILES = BLOC // P
KT = D // 128               # contraction k-tiles
NB = D // 512               # 512-wide PSUM column blocks
LN_EPS = 1e-5

_cache = {}


def _hi_lo_f16(v32):
    hi = v32.astype(np.float16)
    lo = (v32 - hi.astype(np.float32)).astype(np.float16)
    return np.stack([hi, lo], 0)  # [2, D]


def _build(apply_att_affine, apply_ff_affine, has_bias_h, has_bias_s,
           keep_pat, ntiles=NTILES):
    kept = [l for l in range(L) if keep_pat[l]]
    import concourse.bacc as bacc
    import concourse.mybir as mybir
    from concourse.tile import TileContext
    from concourse.masks import make_identity
    from contextlib import ExitStack

    f16 = mybir.dt.float16
    f32 = mybir.dt.float32
    AF = mybir.ActivationFunctionType
    OP = mybir.AluOpType
    AX = mybir.AxisListType

    nc = bacc.Bacc("TRN2", target_bir_lowering=False, debug=False,
                   num_devices=NCORES)

    # ---- DRAM I/O ----
    xbf = nc.dram_tensor("xbf", [BLOC, L * D], f16, kind="ExternalInput")
    xT = nc.dram_tensor("xT", [KT, 128, NTILES, L * P], f16,
                        kind="ExternalInput")
    wts = {
        n: nc.dram_tensor(n, [KT, 128, D], f16, kind="ExternalInput")
        for n in ("wu", "wv", "w1", "w2")
    }
    if has_bias_h:
        bias_h_d = nc.dram_tensor("bias_h", [2, D], f16, kind="ExternalInput")
    if has_bias_s:
        bias_s_d = nc.dram_tensor("bias_s", [2, D], f16, kind="ExternalInput")
    keep_d = nc.dram_tensor("keep", [1, L], f32, kind="ExternalInput")
    mneg_d = nc.dram_tensor("mneg", [1, L], f32, kind="ExternalInput")
    if apply_att_affine:
        attg_d = nc.dram_tensor("attg", [1, D], f16, kind="ExternalInput")
    if apply_ff_affine:
        ffg_d = nc.dram_tensor("ffg", [1, D], f32, kind="ExternalInput")
        ffb_d = nc.dram_tensor("ffb", [1, D], f32, kind="ExternalInput")
    out_d = nc.dram_tensor("out", [BLOC, L, D], f32, kind="ExternalOutput")

    with TileContext(nc) as tc, ExitStack() as ctx:
        const = ctx.enter_context(tc.tile_pool(name="const", bufs=1))
        px = ctx.enter_context(tc.tile_pool(name="px", bufs=2))
        pxT = ctx.enter_context(tc.tile_pool(name="pxT", bufs=2))
        pvh = ctx.enter_context(tc.tile_pool(name="pvh", bufs=2))
        pzh = ctx.enter_context(tc.tile_pool(name="pzh", bufs=2))
        pns = ctx.enter_context(tc.tile_pool(name="pns", bufs=2))
        pn1 = ctx.enter_context(tc.tile_pool(name="pn1", bufs=1))
        psm = ctx.enter_context(tc.tile_pool(name="psm", bufs=3))
        pout = ctx.enter_context(tc.tile_pool(name="pout", bufs=2))
        pps = ctx.enter_context(tc.tile_pool(name="pps", bufs=8, space="PSUM"))

        # ---- constants / weights (resident) ----
        # wu/wv are needed for tile 0 immediately; w1/w2 loads are emitted
        # after tile 0's u/v GEMMs so they don't delay the first matmul.
        w_sb = {n: const.tile([128, KT, D], f16, tag=n, name=n)
                for n in ("wu", "wv", "w1", "w2")}
        # wu/wv loads are emitted inside emit_uv(0), after tile 0's x/xT
        # DMAs, per-k — so the first k=0 matmul only waits on two small DMAs.
        if has_bias_h:
            bias_h = const.tile([2, D], f16, tag="bias_h")
            nc.sync.dma_start(out=bias_h, in_=bias_h_d[:, :])
        if has_bias_s:
            bias_s = const.tile([2, D], f16, tag="bias_s")
            nc.sync.dma_start(out=bias_s, in_=bias_s_d[:, :])
        if has_bias_h or has_bias_s:
            ones2 = const.tile([2, 128], f16, tag="ones2")
            nc.vector.memset(ones2, 1.0)
        ident = const.tile([128, 128], f16, tag="ident")
        make_identity(nc, ident)
        eps_t = const.tile([128, 1], f32, tag="eps")
        nc.vector.memset(eps_t, LN_EPS)
        keep_b = const.tile([128, L], f32, tag="keep")
        nc.gpsimd.dma_start(out=keep_b, in_=keep_d[:, :].to_broadcast([128, L]))
        mneg_b = const.tile([128, L], f32, tag="mneg")
        nc.gpsimd.dma_start(out=mneg_b, in_=mneg_d[:, :].to_broadcast([128, L]))
        if apply_att_affine:
            attg_b = const.tile([128, D], f16, tag="attg")
            nc.gpsimd.dma_start(out=attg_b,
                                in_=attg_d[:, :].to_broadcast([128, D]))
        if apply_ff_affine:
            ffg_b = const.tile([128, D], f32, tag="ffg")
            nc.gpsimd.dma_start(out=ffg_b,
                                in_=ffg_d[:, :].to_broadcast([128, D]))
            ffb_b = const.tile([128, D], f32, tag="ffb")
            nc.gpsimd.dma_start(out=ffb_b,
                                in_=ffb_d[:, :].to_broadcast([128, D]))

        state = {}

        def emit_uv(i):
            """DMA x/xT for tile i, u&v GEMMs, raw scores, v eviction."""
            x_t = px.tile([128, L, D], f16, tag="x")
            nc.sync.dma_start(out=x_t,
                              in_=xbf[i * P:(i + 1) * P, :].rearrange(
                                  "p (l d) -> p l d", l=L))
            # Weight loads ride along per-k with the early tiles' xT DMAs so
            # no matmul ever waits on a multi-MB transfer: wu/wv with tile 0
            # (u/v GEMMs), w1/w2 with tile 1 (needed first by epi(0)).
            if i == 0:
                wnames = ("wu", "wv") + (("w1", "w2") if ntiles == 1 else ())
            elif i == 1:
                wnames = ("w1", "w2")
            else:
                wnames = ()
            xT_t = pxT.tile([128, KT, L * P], f16, tag="xT")
            for k in range(KT):
                nc.sync.dma_start(out=xT_t[:, k, :], in_=xT[k, :, i, :])
                for n in wnames:
                    nc.sync.dma_start(out=w_sb[n][:, k, :],
                                      in_=wts[n][k, :, :])

            v_sb = pvh.tile([128, L, D], f16, tag="vh")
            sc2 = psm.tile([128, L, NB], f32, tag="sc2")
            nc.vector.memset(sc2, 0.0)  # masked l slots stay 0

            for l in kept:
                lhs = [xT_t[:, k, l * P:(l + 1) * P] for k in range(KT)]
                for half, wname in ((0, "wu"), (1, "wv")):
                    for nb in range(NB):
                        ps = pps.tile([128, 512], f32, tag="mm")
                        for k in range(KT):
                            nc.tensor.matmul(
                                ps, lhs[k],
                                w_sb[wname][:, k, nb * 512:(nb + 1) * 512],
                                start=(k == 0), stop=(k == KT - 1))
                        if half == 0:  # u -> scores partial sums
                            scr = psm.tile([128, 512], f16, tag="scr")
                            nc.vector.tensor_mul(
                                scr, x_t[:, l, nb * 512:(nb + 1) * 512], ps)
                            nc.vector.reduce_sum(
                                sc2[:, l, nb:nb + 1], scr, axis=AX.X)
                        else:  # v -> SBUF
                            nc.scalar.activation(
                                out=v_sb[:, l, nb * 512:(nb + 1) * 512],
                                in_=ps, func=AF.Copy)
            state[i] = (x_t, v_sb, sc2)

        def emit_rest(i):
            """Softmax, z, LN1, transposes, FFN, LN2, output for tile i."""
            x_t, v_sb, sc2 = state.pop(i)

            # ---- masked softmax over L ----
            ssum = psm.tile([128, L], f32, tag="ssum")
            nc.vector.tensor_add(ssum, sc2[:, :, 0], sc2[:, :, 1])
            scm = psm.tile([128, L], f32, tag="scm")
            nc.vector.tensor_mul(scm, ssum, keep_b)
            nc.vector.tensor_add(scm, scm, mneg_b)
            mx = psm.tile([128, 1], f32, tag="mx")
            nc.vector.reduce_max(mx, scm, axis=AX.X)
            nmx = psm.tile([128, 1], f32, tag="nmx")
            nc.vector.tensor_scalar(out=nmx, in0=mx, scalar1=-1.0,
                                    scalar2=None, op0=OP.mult)
            e_t = psm.tile([128, L], f32, tag="e")
            nc.scalar.activation(e_t, scm, AF.Exp, bias=nmx, scale=1.0)
            den = psm.tile([128, 1], f32, tag="den")
            nc.vector.reduce_sum(den, e_t, axis=AX.X)
            rden = psm.tile([128, 1], f32, tag="rden")
            nc.vector.reciprocal(rden, den)
            probs = psm.tile([128, L], f32, tag="probs")
            nc.vector.tensor_scalar_mul(probs, e_t, rden)

            # ---- z = probs*v + x (masked l: probs==0 -> z = x) ; LN1 ----
            z_t = pzh.tile([128, L, D], f16, tag="zhT")
            for l in range(L):
                if l in kept:
                    nc.vector.tensor_scalar_mul(z_t[:, l, :], v_sb[:, l, :],
                                                probs[:, l:l + 1])
                    nc.vector.tensor_add(z_t[:, l, :], z_t[:, l, :],
                                         x_t[:, l, :])
                else:
                    nc.any.tensor_copy(out=z_t[:, l, :], in_=x_t[:, l, :])

            mv1 = psm.tile([128, L, 2], f32, tag="mv1")
            st1 = psm.tile([128, 2, 6], f32, tag="st1")
            for l in range(L):
                for c in range(2):
                    nc.vector.bn_stats(st1[:, c, :],
                                       z_t[:, l, c * 512:(c + 1) * 512])
                nc.vector.bn_aggr(mv1[:, l, :], st1)
            # rstd = exp(-0.5*ln(var+eps)): Ln+Exp share one ACT table set
            # with the softmax Exp (Sqrt would force an extra ~1.3us switch).
            la1 = psm.tile([128, L], f32, tag="la1")
            nc.scalar.activation(la1, mv1[:, :, 1], AF.Ln, bias=eps_t,
                                 scale=1.0)
            rstd1 = psm.tile([128, L], f32, tag="rstd1")
            nc.scalar.activation(rstd1, la1, AF.Exp, scale=-0.5)
            n1 = pn1.tile([128, L, D], f16, tag="n1")
            for l in range(L):
                nc.vector.tensor_scalar(
                    out=n1[:, l, :], in0=z_t[:, l, :],
                    scalar1=mv1[:, l, 0:1], scalar2=rstd1[:, l:l + 1],
                    op0=OP.subtract, op1=OP.mult)

            # ---- transpose n1 -> n1T ----
            n1T = pns.tile([128, KT, L * P], f16, tag="n1Ts")
            for l in range(L):
                tp = pps.tile([128, KT * 128], f16, tag="mm")
                for k in range(KT):
                    nc.tensor.transpose(
                        tp[:, k * 128:(k + 1) * 128],
                        n1[:, l, k * 128:(k + 1) * 128], ident)
                nc.any.tensor_copy(
                    out=n1T[:, :, l * P:(l + 1) * P],
                    in_=tp.rearrange("p (a b) -> p a b", a=KT))

            # ---- h = gelu(n1 @ W1 [+ bias_h]) ----
            h_sb = pvh.tile([128, L, D], f16, tag="vh")
            for l in range(L):
                for nb in range(NB):
                    ps = pps.tile([128, 512], f32, tag="mm")
                    if has_bias_h:
                        nc.tensor.matmul(ps, ones2,
                                         bias_h[:, nb * 512:(nb + 1) * 512],
                                         start=True, stop=False)
                    for k in range(KT):
                        nc.tensor.matmul(
                            ps, n1T[:, k, l * P:(l + 1) * P],
                            w_sb["w1"][:, k, nb * 512:(nb + 1) * 512],
                            start=(k == 0 and not has_bias_h),
                            stop=(k == KT - 1))
                    nc.scalar.activation(
                        out=h_sb[:, l, nb * 512:(nb + 1) * 512],
                        in_=ps, func=AF.Gelu)

            # ---- transpose h -> hT ----
            hT = pzh.tile([128, KT, L * P], f16, tag="zhT")
            for l in range(L):
                tp = pps.tile([128, KT * 128], f16, tag="mm")
                for k in range(KT):
                    nc.tensor.transpose(
                        tp[:, k * 128:(k + 1) * 128],
                        h_sb[:, l, k * 128:(k + 1) * 128], ident)
                nc.any.tensor_copy(
                    out=hT[:, :, l * P:(l + 1) * P],
                    in_=tp.rearrange("p (a b) -> p a b", a=KT))

            # ---- ff = h @ W2 [+ bias_s] ; s = ff + n1*att_g ; LN2 ----
            if apply_att_affine:
                n1g = pn1.tile([128, L, D], f16, tag="n1g")
                nc.vector.tensor_mul(n1g, n1, attg_b)
            else:
                n1g = n1
            s_t = pns.tile([128, L, D], f16, tag="n1Ts")
            mv2 = psm.tile([128, L, 2], f32, tag="mv2")
            for l in range(L):
                st2 = psm.tile([128, 2, 6], f32, tag="st2")
                for nb in range(NB):
                    ps = pps.tile([128, 512], f32, tag="mm")
                    if has_bias_s:
                        nc.tensor.matmul(ps, ones2,
                                         bias_s[:, nb * 512:(nb + 1) * 512],
                                         start=True, stop=False)
                    for k in range(KT):
                        nc.tensor.matmul(
                            ps, hT[:, k, l * P:(l + 1) * P],
                            w_sb["w2"][:, k, nb * 512:(nb + 1) * 512],
                            start=(k == 0 and not has_bias_s),
                            stop=(k == KT - 1))
                    nc.vector.tensor_add(
                        s_t[:, l, nb * 512:(nb + 1) * 512], ps,
                        n1g[:, l, nb * 512:(nb + 1) * 512])
                    nc.vector.bn_stats(st2[:, nb, :],
                                       s_t[:, l, nb * 512:(nb + 1) * 512])
                nc.vector.bn_aggr(mv2[:, l, :], st2)
            la2 = psm.tile([128, L], f32, tag="la2")
            nc.scalar.activation(la2, mv2[:, :, 1], AF.Ln, bias=eps_t,
                                 scale=1.0)
            rstd2 = psm.tile([128, L], f32, tag="rstd2")
            nc.scalar.activation(rstd2, la2, AF.Exp, scale=-0.5)

            for l in range(L):
                o_t = pout.tile([128, D], f32, tag="o")
                if apply_ff_affine:
                    n2 = psm.tile([128, D], f16, tag="n2")
                    nc.vector.tensor_scalar(
                        out=n2, in0=s_t[:, l, :],
                        scalar1=mv2[:, l, 0:1], scalar2=rstd2[:, l:l + 1],
                        op0=OP.subtract, op1=OP.mult)
                    nc.vector.tensor_mul(o_t, n2, ffg_b)
                    nc.vector.tensor_add(o_t, o_t, ffb_b)
                else:
                    nc.vector.tensor_scalar(
                        out=o_t, in0=s_t[:, l, :],
                        scalar1=mv2[:, l, 0:1], scalar2=rstd2[:, l:l + 1],
                        op0=OP.subtract, op1=OP.mult)
                nc.sync.dma_start(out=out_d[i * P:(i + 1) * P, l, :], in_=o_t)

        emit_uv(0)
        for i in range(1, ntiles):
            emit_uv(i)
            emit_rest(i - 1)
        emit_rest(ntiles - 1)

    nc.compile()
    return nc


def _prep(x, mask, Wq, Wk, Wv, W1, b1, W2, b2, att_g, att_b, ff_g, ff_b):
    """Host-side preprocessing -> (flags, per-core input maps)."""
    f64 = np.float64
    apply_att_affine = not (np.all(att_g == 1.0) and np.all(att_b == 0.0))
    apply_ff_affine = not (np.all(ff_g == 1.0) and np.all(ff_b == 0.0))

    M = (Wq.astype(f64).T @ Wk.astype(f64)) / np.sqrt(np.float64(D))
    wu = np.ascontiguousarray(M.T).astype(np.float16)          # [d', d]
    wv = np.ascontiguousarray(Wv.T).astype(np.float16)         # [d, e]
    W1g = W1.astype(f64) * att_g.astype(f64)[None, :]
    w1 = np.ascontiguousarray(W1g.T).astype(np.float16)        # [d, e]
    bias_h_f = (b1.astype(f64) + W1.astype(f64) @ att_b.astype(f64)).astype(
        np.float32)
    w2 = np.ascontiguousarray(W2.T).astype(np.float16)         # [e, f]
    bias_s_f = (b2.astype(f64) + att_b.astype(f64)).astype(np.float32)
    has_bias_h = bool(np.any(bias_h_f != 0.0))
    has_bias_s = bool(np.any(bias_s_f != 0.0))

    keep = (np.all(mask != 0, axis=0)).astype(np.float32)[None, :]  # [1, L]
    keep_pat = tuple(bool(k) for k in keep[0])
    mneg = (keep - 1.0) * 30.0

    def wfmt(w):  # [D, D] -> [KT, 128, D]
        return np.ascontiguousarray(w.reshape(KT, 128, D))

    shared = dict(
        wu=wfmt(wu), wv=wfmt(wv), w1=wfmt(w1), w2=wfmt(w2),
        keep=keep, mneg=mneg)
    if has_bias_h:
        shared["bias_h"] = _hi_lo_f16(bias_h_f)
    if has_bias_s:
        shared["bias_s"] = _hi_lo_f16(bias_s_f)
    if apply_att_affine:
        shared["attg"] = att_g.astype(np.float16)[None, :]
    if apply_ff_affine:
        shared["ffg"] = ff_g.astype(np.float32)[None, :]
        shared["ffb"] = ff_b.astype(np.float32)[None, :]

    x16 = x.astype(np.float16)
    in_maps = []
    for c in range(NCORES):
        xc = x16[c * BLOC:(c + 1) * BLOC]                      # [BLOC, L, D]
        xbf = np.ascontiguousarray(xc.reshape(BLOC, L * D))
        # [i, s, l, k, dk] -> [k, dk, i, l, s]
        xTc = np.ascontiguousarray(
            xc.reshape(NTILES, P, L, KT, 128).transpose(3, 4, 0, 2, 1)
        ).reshape(KT, 128, NTILES, L * P)
        in_maps.append(dict(shared, xbf=xbf, xT=xTc))
    flags = (apply_att_affine, apply_ff_affine, has_bias_h, has_bias_s,
             keep_pat)
    return flags, in_maps


def kernel(**inputs):
    from concourse.bass_utils import run_bass_kernel_spmd

    flags, in_maps = _prep(**inputs)
    if flags not in _cache:
        _cache[flags] = _build(*flags)
    nc = _cache[flags]
    res = run_bass_kernel_spmd(nc, in_maps, core_ids=list(range(NCORES)))
    out = np.concatenate([r["out"] for r in res.results], axis=0)
    return out.astype(np.float32)
